# revision 58
# baseline (speedup 1.0000x reference)
"""MultiHead Differential Attention on 8 Trainium2 NeuronCores.

Sharding: data-parallel over batch (B=2), tensor-parallel over heads
(16 heads -> 4 per core).  Core c handles batch c//4, heads (c%4)*4..+4.

Device layout is fully "transposed" (S^T = [k, q] orientation) so that no
on-device transposes are ever needed:
  - projections compute Q^T, K^T directly ([2*Dh, seq]); V naturally [seq, dv]
  - S^T[k,q] = K^T.T @ Q^T  (contraction over d on partitions)
  - softmax row-sums come from an all-ones stationary matmul (M=128 -> the
    sums arrive pre-broadcast across partitions)
  - PV keeps V stationary: O^T[dv,q] accumulates over k-chunks
  - the differential combine d = o1/g - (lam/g)*(s1/s2)*o2 uses a fast DVE
    reciprocal; the leftover s1/g column scale washes out in the RMS norm
    (g = max(|lam|,1) keeps d^2 inside bf16 range)
  - out-proj streams O^T with Wo stationary, producing OUT^T which the host
    transposes and sum-reduces.

The per-qc pipeline is fused (projections for qc, then attention for qc,
then the output projection for qc).  Projection groups compute two outputs
per PSUM tile and drain with one wide copy; out-projection reuses the
o1/o2 PSUM banks (free after the epilogue) so the S-group staging slots
are never blocked behind the epilogue chain.

Projections run K -> V -> Q so the kt drains (the attention stationaries,
consumed first) get the whole projection phase as slack.  The two S
components of each k-chunk are emitted back-to-back into one PSUM tile:
their stationaries occupy disjoint PE row groups (partitions 0:64 /
64:128), so the hardware overlaps the two 64-contraction matmuls via row
tiling, spanning ~one matmul duration instead of two.
"""
import numpy as np
import ml_dtypes
from contextlib import ExitStack

import concourse.bass as bass
import concourse.mybir as mybir
import concourse.tile as tile
from concourse import bacc
from concourse.bass_utils import run_bass_kernel_spmd

BF16 = mybir.dt.bfloat16
F32 = mybir.dt.float32
AF = mybir.ActivationFunctionType
ALU = mybir.AluOpType

D_MODEL = 1024
H = 16
DH = 64          # head dim per component
HD = 2 * DH      # 128, per-head width of Q/K/V
N = 2048         # sequence length
B = 2
HPC = 4          # heads per core
LAMBDA_INIT = 0.8
EPS = 1e-5
SCALING = 1.0 / np.sqrt(DH)

MC = D_MODEL // 128   # 8 contraction chunks for projections
QC = 4                # q chunks of 512
KCQ = 4               # k-chunks (128) per q chunk
NKC = 16              # total k chunks

_cache = {}


def _patch_act_tables():
    """Force Exp and Ln to resolve to the single set that contains both,
    so alternating Exp/Ln never reloads activation tables."""
    import concourse.bacc as bacc_mod
    import concourse.hw_specs as hw_specs_mod
    if getattr(bacc_mod, "_act_tables_patched", False):
        return
    orig = hw_specs_mod.get_activation_tables

    def patched(arch):
        t = orig(arch)
        for name, fns in t.items():
            if name != "natural_log_exp_and_others":
                fns.discard(AF.Exp)
                fns.discard(AF.Ln)
        return t

    bacc_mod.get_activation_tables = patched
    bacc_mod._act_tables_patched = True


def _patch_sched_dve_pessimism():
    """Scheduling-only: the Tile scheduler's DVE timing is ~3x optimistic
    versus hardware (measured 402ns for a [128,512] bf16 add vs 133ns
    modeled), which makes it order DVE-dependent matmuls (softmax-sum
    accumulations) too early in the in-order PE stream, stalling the PE.
    Pessimise the modeled DVE clock so the schedule interleaves
    independent matmuls first.  Hardware execution is unaffected."""
    import concourse.hw_specs as hw_specs_mod
    spec = hw_specs_mod.TRN2Spec
    if getattr(spec, "_dve_pessimism", False):
        return
    spec.CYCLE_T = {**spec.CYCLE_T,
                    mybir.EngineType.DVE: 1e9 / 0.40e9}
    spec._dve_pessimism = True


def _build():
    _patch_act_tables()
    nc = bacc.Bacc("TRN2", target_bir_lowering=False, debug=False)

    xt_d = nc.dram_tensor("xt", [128, MC, N], BF16, kind="ExternalInput").ap()
    wq_d = nc.dram_tensor("wq", [128, MC, HPC * HD], BF16, kind="ExternalInput").ap()
    wk_d = nc.dram_tensor("wk", [128, MC, HPC * HD], BF16, kind="ExternalInput").ap()
    wv_d = nc.dram_tensor("wv", [128, MC, HPC * HD], BF16, kind="ExternalInput").ap()
    wo_d = nc.dram_tensor("wo", [128, HPC, 8, 128], BF16, kind="ExternalInput").ap()
    lam_d = nc.dram_tensor("lam", [128, 2 * HPC], F32, kind="ExternalInput").ap()
    msk_d = nc.dram_tensor("msk", [128, KCQ, 512], BF16, kind="ExternalInput").ap()
    out_d = nc.dram_tensor("outT", [D_MODEL, N], BF16, kind="ExternalOutput").ap()

    with tile.TileContext(nc) as tc, ExitStack() as ctx:
        # ---- long-lived tiles
        keep = ctx.enter_context(tc.tile_pool(name="keep", bufs=1))
        # round-major layout [p, qc, head, 512]: each projection drain then
        # writes one CONTIGUOUS byte range, so its bounding box cannot alias
        # other heads'/rounds' regions and create false LDWEIGHTS deps.
        qt = keep.tile([128, QC, HPC, 512], BF16, tag="qt")
        kt = keep.tile([128, QC, HPC, 512], BF16, tag="kt")
        vb = keep.tile([128, NKC, 512], BF16, tag="vb")
        otf = [keep.tile([128, N], BF16, tag=f"otf{h}", name=f"otf{h}") for h in range(HPC)]
        lam_t = keep.tile([128, 2 * HPC], F32, tag="lam")
        # only the j=0 triangle block of the host mask is ever used
        msk_t = keep.tile([128, 128], BF16, tag="msk")
        ones_t = keep.tile([128, 128], BF16, tag="ones")
        eps_t = keep.tile([128, 1], F32, tag="eps")
        wo_t = keep.tile([128, HPC, 8, 128], BF16, tag="wo")

        nc.gpsimd.memset(ones_t[:], 1.0)
        nc.gpsimd.memset(eps_t[:], float(EPS))

        pj = ctx.enter_context(tc.tile_pool(name="proj", bufs=1))
        psum = ctx.enter_context(tc.tile_pool(name="psum", bufs=1, space="PSUM"))
        at = ctx.enter_context(tc.tile_pool(name="att", bufs=2))
        ep = ctx.enter_context(tc.tile_pool(name="esb", bufs=2))
        osb = ctx.enter_context(tc.tile_pool(name="osb", bufs=2))

        xtb = pj.tile([128, MC, N], BF16, tag="xtb")
        wqb = pj.tile([128, MC, HPC * HD], BF16, tag="wqb")
        wkb = pj.tile([128, MC, HPC * HD], BF16, tag="wkb")
        wvb = pj.tile([128, MC, HPC * HD], BF16, tag="wvb")
        # load X^T at per-(mc, seq-chunk) granularity.  The first DMA wave
        # interleaves exactly what the first K-projection group consumes (X^T
        # mc-chunk paired with the matching Wk chunk, in mc order) so the PE
        # can start on mc=0 while later chunks stream; Wv then Wq follow in
        # the order the V/Q projection groups reach them.
        # the first K-projection wave issues from the (startup-idle) Act and
        # DVE sequencers in parallel with Sync issuing the rest — the Sync
        # engine's ~610ns per-issue rate was pacing the kernel start
        for mc in range(MC):
            if mc == 0:
                # quarters on separate DMA queues: per-queue bandwidth
                # (~20GB/s) paces the arrival of the first quantum's operands
                for qr in range(4):
                    nc.scalar.dma_start(xtb[:, mc, qr * 128:(qr + 1) * 128],
                                        xt_d[:, mc, qr * 128:(qr + 1) * 128])
                for qr in range(4):
                    nc.sync.dma_start(wkb[:, mc, qr * 128:(qr + 1) * 128],
                                      wk_d[:, mc, qr * 128:(qr + 1) * 128])
            elif mc < 3:
                nc.scalar.dma_start(xtb[:, mc, 0:256], xt_d[:, mc, 0:256])
                nc.scalar.dma_start(xtb[:, mc, 256:512], xt_d[:, mc, 256:512])
                nc.sync.dma_start(wkb[:, mc, 0:256], wk_d[:, mc, 0:256])
                nc.sync.dma_start(wkb[:, mc, 256:512], wk_d[:, mc, 256:512])
            else:
                nc.scalar.dma_start(xtb[:, mc, 0:512], xt_d[:, mc, 0:512])
                nc.sync.dma_start(wkb[:, mc, :], wk_d[:, mc, :])
        for mc in range(MC):
            nc.sync.dma_start(wvb[:, mc, :], wv_d[:, mc, :])
        for mc in range(MC):
            nc.sync.dma_start(wqb[:, mc, :], wq_d[:, mc, :])
        nc.sync.dma_start(msk_t[:], msk_d[:, 0, 0:128])
        nc.sync.dma_start(lam_t[:], lam_d[:])
        # round-1 X^T ahead of wo: the bulk proj(1) quanta at round 0's end
        # are the earliest consumer; wo isn't needed until outproj(0)
        for mc in range(MC):
            nc.scalar.dma_start(xtb[:, mc, 512:1024], xt_d[:, mc, 512:1024])
        nc.sync.dma_start(wo_t[:], wo_d[:])
        for qch in range(2, QC):
            for mc in range(MC):
                nc.sync.dma_start(xtb[:, mc, qch * 512:(qch + 1) * 512],
                                  xt_d[:, mc, qch * 512:(qch + 1) * 512])

        # PSUM rings: 8 banks total = sg 2x2 + pp 2x1 + o1 + o2.
        PSB = {"sg": 2, "pp": 2}

        def drain(dst, src):
            # all PSUM drains on the DVE: the Act engine is the busier of
            # the two (it owns every exp), so it keeps only activations
            nc.vector.tensor_copy(dst, src)

        # ---- projection / out-projection work, chopped into "quanta" of
        # two 512-wide matmuls each.  Quanta are interleaved into the
        # attention kc loops as PE filler: the attention stream is paced by
        # the Act exp pipeline (one [128,1024] exp per k-chunk takes ~1040ns
        # vs ~810ns of PE work), so without filler the PE idles at every
        # sg-slot rotation.  tag selects the PSUM ring ("pp" during
        # attention, "sg" only when the attention stream is quiet).

        def proj_v_half(r, sc, tag):
            st = {}

            def quantum(mcp, st=st, sc=sc, tag=tag):
                if mcp == 0:
                    st['ps'] = psum.tile([128, 512], F32, tag=tag,
                                         name="pjv", bufs=PSB[tag])
                ps = st['ps']
                for mc in (2 * mcp, 2 * mcp + 1):
                    nc.tensor.matmul(
                        ps[:], xtb[:, mc, sc * 128:(sc + 1) * 128],
                        wvb[:, mc, :],
                        start=(mc == 0), stop=(mc == MC - 1))
                if mcp == 3:
                    drain(vb[:, sc, :], ps[:])

            return [lambda m=m: quantum(m) for m in range(4)]

        def proj_qk_half(r, wsrc, dst, hh, tag):
            st = {}

            def quantum(mcp, st=st, r=r, hh=hh, tag=tag):
                if mcp == 0:
                    st['ps'] = psum.tile([128, 512], F32, tag=tag,
                                         name="pjqk", bufs=PSB[tag])
                ps = st['ps']
                for mc in (2 * mcp, 2 * mcp + 1):
                    nc.tensor.matmul(
                        ps[:], wsrc[:, mc, hh * HD:(hh + 1) * HD],
                        xtb[:, mc, r * 512:(r + 1) * 512],
                        start=(mc == 0), stop=(mc == MC - 1))
                if mcp == 3:
                    drain(dst[:, r, hh, :], ps[:])

            return [lambda m=m: quantum(m) for m in range(4)]

        def proj_quanta(r, tags):
            # K first: the attention S-matmuls' stationaries (kt) are the
            # first thing round r's heads consume; V next (PV stationaries);
            # Q (the S moving operand, needed per-head) last.
            # Returns a list of GROUPS (each one PSUM tenancy).
            units = ([('k', hh) for hh in range(HPC)]
                     + [('v', KCQ * r + i) for i in range(KCQ)]
                     + [('q', hh) for hh in range(HPC)])
            out = []
            for ui, (kind, a) in enumerate(units):
                tag = tags[ui % len(tags)]
                if kind == 'k':
                    out.append(proj_qk_half(r, wkb, kt, a, tag))
                elif kind == 'v':
                    out.append(proj_v_half(r, a, tag))
                else:
                    out.append(proj_qk_half(r, wqb, qt, a, tag))
            return out

        def outproj_half(rr, oc, tag):
            # output chunk oc of round rr, accumulated over the four heads;
            # quantum = two head matmuls.
            st = {}

            def quantum(hp, st=st, rr=rr, oc=oc, tag=tag):
                if hp == 0:
                    st['ps'] = psum.tile([128, 512], F32, tag=tag,
                                         name="ops", bufs=PSB[tag])
                ps = st['ps']
                for h in (2 * hp, 2 * hp + 1):
                    nc.tensor.matmul(
                        ps[:], wo_t[:, h, oc, :],
                        otf[h][:, rr * 512:(rr + 1) * 512],
                        start=(h == 0), stop=(h == HPC - 1))
                if hp == 1:
                    ob = osb.tile([128, 512], BF16, tag="ob")
                    drain(ob[:], ps[:])
                    nc.sync.dma_start(
                        out_d[oc * 128:(oc + 1) * 128,
                              rr * 512:(rr + 1) * 512],
                        ob[:])

            return [lambda m=m: quantum(m) for m in range(2)]

        class Filler:
            """Dribbles queued quanta (grouped by PSUM tenancy) into the
            attention emission at a fixed per-kc-event rate; boost() places
            a few right before known stall points; finish_group() completes
            the open group so the pp ring has no long-lived tenant when the
            softmax-sum tiles claim their slots; flush() emits the rest."""

            def __init__(self, groups, rate):
                self.groups = [list(g) for g in groups]
                self.gi = 0
                self.qi = 0
                self.rate = rate
                self.credit = 0.0

            def _pop1(self):
                while self.gi < len(self.groups):
                    g = self.groups[self.gi]
                    if self.qi < len(g):
                        q = g[self.qi]
                        self.qi += 1
                        q()
                        return True
                    self.gi += 1
                    self.qi = 0
                return False

            def tick(self):
                self.credit += self.rate
                while self.credit >= 1.0 and self._pop1():
                    self.credit -= 1.0
                self.credit = min(self.credit, 4.0)

            def boost(self, n):
                for _ in range(n):
                    if not self._pop1():
                        break

            def finish_group(self):
                if self.gi < len(self.groups) and self.qi > 0:
                    g = self.groups[self.gi]
                    while self.qi < len(g):
                        g[self.qi]()
                        self.qi += 1
                    self.gi += 1
                    self.qi = 0

            def flush(self):
                while self._pop1():
                    pass

        def attn_head(qc, h, tick, boost, finish):
            nkc = KCQ * qc + KCQ  # k chunks in play
            q0 = qc * 512
            # filler ahead of the head's first S matmuls: they wait for the
            # previous head's exp/epilogue to free their PSUM slots, and the
            # in-order PE can't pull later work past them
            boost(3)
            o1 = psum.tile([128, 512], F32, tag="o1", name="o1")
            o2 = psum.tile([128, 512], F32, tag="o2", name="o2")
            pending = []
            # deferred softmax-sum contributions: (bf16 tile ap, w0) per
            # comp; full chunks fold pairwise then pairs into quads on the
            # DVE, diagonal chunks contribute directly.
            sums = ([], [])
            pair_hold = [None]  # e tile awaiting its pair partner
            quad_hold = [None]  # pair tile awaiting its quad partner

            def emit_pv(item):
                e, kc, w0 = item
                st = (kc == 0)
                sp = (kc == nkc - 1)
                nc.tensor.matmul(
                    o1[:, w0:512], vb[:, kc, h * HD:(h + 1) * HD],
                    e[:, 0, w0:512], start=st, stop=sp)
                nc.tensor.matmul(
                    o2[:, w0:512], vb[:, kc, h * HD:(h + 1) * HD],
                    e[:, 1, w0:512], start=st, stop=sp)

            for kc in range(nkc):
                j = kc - KCQ * qc
                w0 = max(0, 128 * j)  # first valid col of chunk
                # Both components of one k-chunk go to one fresh PSUM tile,
                # emitted back-to-back: the stationaries sit in disjoint PE
                # row groups (partitions 0:64 / 64:128), so the hardware runs
                # the two 64-contraction matmuls CONCURRENTLY (row tiling) —
                # the pair spans ~one matmul duration, not two.
                ps = psum.tile([128, 2, 512], F32, tag="sg", name="s12",
                               bufs=PSB["sg"])
                qcc, c0 = kc // KCQ, (kc % KCQ) * 128
                nc.tensor.matmul(
                    ps[:, 0, w0:512], kt[0:64, qcc, h, c0:c0 + 128],
                    qt[0:64, qc, h, w0:512], start=True, stop=True)
                nc.tensor.matmul(
                    ps[:, 1, w0:512], kt[64:128, qcc, h, c0:c0 + 128],
                    qt[64:128, qc, h, w0:512], start=True, stop=True)
                e = ep.tile([128, 2, 512], BF16, tag="e", name="e", bufs=10)
                if j >= 2:
                    # narrow chunk: skip the dead prefix on the Act engine
                    # (two contiguous activations — a single strided-AP one
                    # measured ~45% slower on hardware)
                    nc.scalar.activation(
                        e[:, 0, w0:512], ps[:, 0, w0:512],
                        AF.Exp, scale=float(SCALING))
                    nc.scalar.activation(
                        e[:, 1, w0:512], ps[:, 1, w0:512],
                        AF.Exp, scale=float(SCALING))
                else:
                    nc.scalar.activation(
                        e[:].rearrange("p a b -> p (a b)"),
                        ps[:].rearrange("p a b -> p (a b)"),
                        AF.Exp, scale=float(SCALING))
                if j >= 0:  # triangle mask on the diagonal block
                    for c in (0, 1):
                        nc.vector.tensor_mul(
                            e[:, c, w0:w0 + 128], e[:, c, w0:w0 + 128],
                            msk_t[:, 0:128])
                    sums[0].append((e[:, 0, w0:512], w0))
                    sums[1].append((e[:, 1, w0:512], w0))
                elif pair_hold[0] is None:
                    pair_hold[0] = e
                else:
                    e_prev, pair_hold[0] = pair_hold[0], None
                    pr = ep.tile([128, 2, 512], BF16, tag="pr", name="pr",
                                 bufs=4)
                    nc.vector.tensor_add(pr[:, 0, :], e_prev[:, 0, :],
                                         e[:, 0, :])
                    nc.vector.tensor_add(pr[:, 1, :], e_prev[:, 1, :],
                                         e[:, 1, :])
                    if quad_hold[0] is None:
                        quad_hold[0] = pr
                    else:
                        pr_prev, quad_hold[0] = quad_hold[0], None
                        qd = ep.tile([128, 2, 512], BF16, tag="qd", name="qd",
                                     bufs=3)
                        nc.vector.tensor_add(qd[:, 0, :], pr_prev[:, 0, :],
                                             pr[:, 0, :])
                        nc.vector.tensor_add(qd[:, 1, :], pr_prev[:, 1, :],
                                             pr[:, 1, :])
                        sums[0].append((qd[:, 0, :], 0))
                        sums[1].append((qd[:, 1, :], 0))
                pending.append((e, kc, w0))
                if len(pending) > 6:
                    emit_pv(pending.pop(0))
                tick()
            if quad_hold[0] is not None:
                pr, quad_hold[0] = quad_hold[0], None
                sums[0].append((pr[:, 0, :], 0))
                sums[1].append((pr[:, 1, :], 0))
            if pair_hold[0] is not None:
                e_left, pair_hold[0] = pair_hold[0], None
                sums[0].append((e_left[:, 0, :], 0))
                sums[1].append((e_left[:, 1, :], 0))
            while pending:
                emit_pv(pending.pop(0))
            # filler right before the sum matmuls: their moving operands
            # come off the exp/fold pipeline, so the PE otherwise stalls
            boost(2)
            # close any open filler group, then the softmax-sum tiles take
            # the two pp slots (keeping both sg slots for the S stream)
            finish()
            ssum1 = psum.tile([128, 512], F32, tag="pp", name="ssum1",
                              bufs=PSB["pp"])
            ssum2 = psum.tile([128, 512], F32, tag="pp", name="ssum2",
                              bufs=PSB["pp"])
            for half, contribs in enumerate(sums):
                dst = ssum1 if half == 0 else ssum2
                for ci, (src, w0c) in enumerate(contribs):
                    nc.tensor.matmul(dst[:, w0c:512], ones_t[:], src,
                                     start=(ci == 0),
                                     stop=(ci == len(contribs) - 1))
            # ---- epilogue: d = o1/g - (lam/g)*(s1/s2)*o2; the s1/g
            # column scale cancels in the RMS norm.  |lam/g| <= 1 keeps
            # d^2 inside bf16 range.  Division via fast DVE reciprocal.
            r2 = at.tile([128, 512], F32, tag="r2")
            nc.vector.reciprocal_approx_fast(out=r2[:], in_=ssum2[:])
            w = at.tile([128, 512], F32, tag="w")
            nc.vector.scalar_tensor_tensor(
                w[:], ssum1[:], lam_t[:, h:h + 1], r2[:],
                ALU.mult, ALU.mult)
            t = at.tile([128, 512], F32, tag="t", bufs=1)
            nc.vector.tensor_mul(t[:], o2[:], w[:])
            d = at.tile([128, 512], BF16, tag="d")
            nc.vector.scalar_tensor_tensor(
                d[:], o1[:], lam_t[:, HPC + h:HPC + h + 1], t[:],
                ALU.mult, ALU.subtract)
            osq = at.tile([128, 512], BF16, tag="osq")
            nc.vector.tensor_mul(osq[:], d[:], d[:])
            # the o1 bank is free right here (d has consumed it)
            ssq = psum.tile([128, 512], F32, tag="o1", name="ssq")
            nc.tensor.matmul(ssq[:], ones_t[:], osq[:],
                             start=True, stop=True)
            lnv = at.tile([128, 512], F32, tag="lnv", bufs=1)
            nc.scalar.activation(lnv[:], ssq[:], AF.Ln,
                                 scale=float(1.0 / HD), bias=eps_t[:])
            rr = at.tile([128, 512], BF16, tag="rr")
            nc.scalar.activation(rr[:], lnv[:], AF.Exp, scale=-0.5)
            nc.vector.tensor_mul(otf[h][:, q0:q0 + 512], d[:], rr[:])

        # ---- emission schedule: round 0's projections run upfront; after
        # that, round qc+1's projection quanta are dribbled INTO round qc's
        # attention loops (the attention stream is Act-paced, so the PE has
        # slack there), out-projections for rounds 0/1 go at their round
        # ends, round 2's out-projection fills round 3's attention, and
        # round 3's out-projection is the tail.
        for g in proj_quanta(0, ("pp",)):
            for q in g:
                q()
        for qc in range(QC):
            events = HPC * (KCQ * qc + KCQ)
            if qc < QC - 1:
                # round 0: the round-1 X^T chunks are still in flight on the
                # DMA queues during round-0 attention, so dribbling proj(1)
                # quanta in would stall the PE on data — emit them all in
                # the round-end bulk instead.  The uniform rate leaves ~20
                # quanta per round for the targeted stall-point boosts.
                rate = 0.0 if qc == 0 else 28.0 / events
                fill = Filler(proj_quanta(qc + 1, ("pp",)), rate)
            else:
                # boosts alone place the round-2 out-projection quanta at
                # the stall points
                fill = Filler([outproj_half(3 - 1, oc, "pp")
                               for oc in range(8)], 0.0)
            for h in range(HPC):
                # round 0, head 0 is the only window where filler data
                # (round-1 X^T) hasn't landed yet — no boosts there.
                # Last round: ration the 16 remaining quanta (4 per head)
                # so head 3's stall points aren't starved.
                if qc == 0 and h == 0:
                    boost = (lambda n: None)
                elif qc == QC - 1:
                    bb = [4]

                    def boost(n, bb=bb):
                        k = min(n, bb[0])
                        if k > 0:
                            bb[0] -= k
                            fill.boost(k)
                else:
                    boost = fill.boost
                attn_head(qc, h, fill.tick, boost, fill.finish_group)
            fill.flush()
            if qc < QC - 2:
                for oc in range(8):
                    for q in outproj_half(qc, oc, "pp"):
                        q()
            elif qc == QC - 1:
                for oc in range(8):
                    for q in outproj_half(qc, oc, "pp"):
                        q()

    nc.compile()
    return nc


def _prep_inputs(X, Wq, Wk, Wv, Wo, lambda_q1, lambda_k1, lambda_q2,
                 lambda_k2, rms_scale):
    f32 = np.float32
    bf16 = ml_dtypes.bfloat16
    X = np.asarray(X, f32)
    Wq = np.asarray(Wq, f32)
    Wk = np.asarray(Wk, f32)
    Wv = np.asarray(Wv, f32)
    Wo = np.asarray(Wo, f32)
    lam = (np.exp(np.sum(np.asarray(lambda_q1, f32) * np.asarray(lambda_k1, f32), -1))
           - np.exp(np.sum(np.asarray(lambda_q2, f32) * np.asarray(lambda_k2, f32), -1))
           + f32(LAMBDA_INIT)).astype(f32)  # [H]
    # fold rms_scale and (1-lambda_init) into Wo
    wo_f = (Wo.reshape(H, HD, D_MODEL)
            * np.asarray(rms_scale, f32)[None, :, None]
            * f32(1.0 - LAMBDA_INIT)).astype(f32)

    # causal masks for the 4 diagonal-region chunk offsets
    msk = np.zeros((128, KCQ, 512), f32)
    kk = np.arange(128)[:, None]
    cc = np.arange(512)[None, :]
    for j in range(KCQ):
        msk[:, j, :] = (cc >= 128 * j + kk).astype(f32)

    in_maps = []
    for c in range(8):
        b, hg = divmod(c, 4)
        xt = X[b].T.reshape(MC, 128, N).transpose(1, 0, 2)  # [128, MC, N]
        sl = slice(hg * HPC * HD, (hg + 1) * HPC * HD)
        wq = Wq[:, sl].reshape(MC, 128, HPC * HD).transpose(1, 0, 2)
        wk = Wk[:, sl].reshape(MC, 128, HPC * HD).transpose(1, 0, 2)
        wv = Wv[:, sl].reshape(MC, 128, HPC * HD).transpose(1, 0, 2)
        wo = wo_f[hg * HPC:(hg + 1) * HPC].reshape(HPC, HD, 8, 128).transpose(1, 0, 2, 3)
        lv = lam[hg * HPC:(hg + 1) * HPC]
        g = np.maximum(np.abs(lv), f32(1.0)).astype(f32)
        lam_row = np.concatenate([lv / g, 1.0 / g]).astype(f32)
        lam_bc = np.broadcast_to(lam_row[None, :], (128, 2 * HPC))
        in_maps.append({
            "xt": np.ascontiguousarray(xt).astype(bf16),
            "wq": np.ascontiguousarray(wq).astype(bf16),
            "wk": np.ascontiguousarray(wk).astype(bf16),
            "wv": np.ascontiguousarray(wv).astype(bf16),
            "wo": np.ascontiguousarray(wo).astype(bf16),
            "lam": np.ascontiguousarray(lam_bc.astype(f32)),
            "msk": msk.astype(bf16),
        })
    return in_maps


def kernel(X, Wq, Wk, Wv, Wo, lambda_q1, lambda_k1, lambda_q2, lambda_k2,
           rms_scale, _trace=False):
    if "nc" not in _cache:
        _cache["nc"] = _build()
    nc = _cache["nc"]
    in_maps = _prep_inputs(X, Wq, Wk, Wv, Wo, lambda_q1, lambda_k1,
                           lambda_q2, lambda_k2, rms_scale)
    res = run_bass_kernel_spmd(nc, in_maps, list(range(8)), trace=_trace)
    out = np.zeros((B, N, D_MODEL), np.float32)
    for c in range(8):
        b = c // 4
        out[b] += res.results[c]["outT"].T.astype(np.float32)
    _cache["last_exec_ns"] = res.exec_time_ns
    _cache["last_res"] = res
    return out



# revision 59
# speedup vs baseline: 1.0067x; 1.0067x over previous
"""MultiHead Differential Attention on 8 Trainium2 NeuronCores.

Sharding: data-parallel over batch (B=2), tensor-parallel over heads
(16 heads -> 4 per core).  Core c handles batch c//4, heads (c%4)*4..+4.

Device layout is fully "transposed" (S^T = [k, q] orientation) so that no
on-device transposes are ever needed:
  - projections compute Q^T, K^T directly ([2*Dh, seq]); V naturally [seq, dv]
  - S^T[k,q] = K^T.T @ Q^T  (contraction over d on partitions)
  - softmax row-sums come from an all-ones stationary matmul (M=128 -> the
    sums arrive pre-broadcast across partitions)
  - PV keeps V stationary: O^T[dv,q] accumulates over k-chunks
  - the differential combine d = o1/g - (lam/g)*(s1/s2)*o2 uses a fast DVE
    reciprocal; the leftover s1/g column scale washes out in the RMS norm
    (g = max(|lam|,1) keeps d^2 inside bf16 range)
  - out-proj streams O^T with Wo stationary, producing OUT^T which the host
    transposes and sum-reduces.

The per-qc pipeline is fused (projections for qc, then attention for qc,
then the output projection for qc).  Projection groups compute two outputs
per PSUM tile and drain with one wide copy; out-projection reuses the
o1/o2 PSUM banks (free after the epilogue) so the S-group staging slots
are never blocked behind the epilogue chain.

Projections run K -> V -> Q so the kt drains (the attention stationaries,
consumed first) get the whole projection phase as slack.  The two S
components of each k-chunk are emitted back-to-back into one PSUM tile:
their stationaries occupy disjoint PE row groups (partitions 0:64 /
64:128), so the hardware overlaps the two 64-contraction matmuls via row
tiling, spanning ~one matmul duration instead of two.
"""
import numpy as np
import ml_dtypes
from contextlib import ExitStack

import concourse.bass as bass
import concourse.mybir as mybir
import concourse.tile as tile
from concourse import bacc
from concourse.bass_utils import run_bass_kernel_spmd

BF16 = mybir.dt.bfloat16
F32 = mybir.dt.float32
AF = mybir.ActivationFunctionType
ALU = mybir.AluOpType

D_MODEL = 1024
H = 16
DH = 64          # head dim per component
HD = 2 * DH      # 128, per-head width of Q/K/V
N = 2048         # sequence length
B = 2
HPC = 4          # heads per core
LAMBDA_INIT = 0.8
EPS = 1e-5
SCALING = 1.0 / np.sqrt(DH)

MC = D_MODEL // 128   # 8 contraction chunks for projections
QC = 4                # q chunks of 512
KCQ = 4               # k-chunks (128) per q chunk
NKC = 16              # total k chunks

_cache = {}


def _patch_act_tables():
    """Force Exp and Ln to resolve to the single set that contains both,
    so alternating Exp/Ln never reloads activation tables."""
    import concourse.bacc as bacc_mod
    import concourse.hw_specs as hw_specs_mod
    if getattr(bacc_mod, "_act_tables_patched", False):
        return
    orig = hw_specs_mod.get_activation_tables

    def patched(arch):
        t = orig(arch)
        for name, fns in t.items():
            if name != "natural_log_exp_and_others":
                fns.discard(AF.Exp)
                fns.discard(AF.Ln)
        return t

    bacc_mod.get_activation_tables = patched
    bacc_mod._act_tables_patched = True


def _patch_sched_dve_pessimism():
    """Scheduling-only: the Tile scheduler's DVE timing is ~3x optimistic
    versus hardware (measured 402ns for a [128,512] bf16 add vs 133ns
    modeled), which makes it order DVE-dependent matmuls (softmax-sum
    accumulations) too early in the in-order PE stream, stalling the PE.
    Pessimise the modeled DVE clock so the schedule interleaves
    independent matmuls first.  Hardware execution is unaffected."""
    import concourse.hw_specs as hw_specs_mod
    spec = hw_specs_mod.TRN2Spec
    if getattr(spec, "_dve_pessimism", False):
        return
    spec.CYCLE_T = {**spec.CYCLE_T,
                    mybir.EngineType.DVE: 1e9 / 0.40e9}
    spec._dve_pessimism = True


def _build():
    _patch_act_tables()
    nc = bacc.Bacc("TRN2", target_bir_lowering=False, debug=False)

    xt_d = nc.dram_tensor("xt", [128, MC, N], BF16, kind="ExternalInput").ap()
    wq_d = nc.dram_tensor("wq", [128, MC, HPC * HD], BF16, kind="ExternalInput").ap()
    wk_d = nc.dram_tensor("wk", [128, MC, HPC * HD], BF16, kind="ExternalInput").ap()
    wv_d = nc.dram_tensor("wv", [128, MC, HPC * HD], BF16, kind="ExternalInput").ap()
    wo_d = nc.dram_tensor("wo", [128, HPC, 8, 128], BF16, kind="ExternalInput").ap()
    lam_d = nc.dram_tensor("lam", [128, 2 * HPC], F32, kind="ExternalInput").ap()
    msk_d = nc.dram_tensor("msk", [128, KCQ, 512], BF16, kind="ExternalInput").ap()
    out_d = nc.dram_tensor("outT", [D_MODEL, N], BF16, kind="ExternalOutput").ap()

    with tile.TileContext(nc) as tc, ExitStack() as ctx:
        # ---- long-lived tiles
        keep = ctx.enter_context(tc.tile_pool(name="keep", bufs=1))
        # round-major layout [p, qc, head, 512]: each projection drain then
        # writes one CONTIGUOUS byte range, so its bounding box cannot alias
        # other heads'/rounds' regions and create false LDWEIGHTS deps.
        qt = keep.tile([128, QC, HPC, 512], BF16, tag="qt")
        kt = keep.tile([128, QC, HPC, 512], BF16, tag="kt")
        vb = keep.tile([128, NKC, 512], BF16, tag="vb")
        otf = [keep.tile([128, N], BF16, tag=f"otf{h}", name=f"otf{h}") for h in range(HPC)]
        lam_t = keep.tile([128, 2 * HPC], F32, tag="lam")
        # only the j=0 triangle block of the host mask is ever used
        msk_t = keep.tile([128, 128], BF16, tag="msk")
        ones_t = keep.tile([128, 128], BF16, tag="ones")
        eps_t = keep.tile([128, 1], F32, tag="eps")
        wo_t = keep.tile([128, HPC, 8, 128], BF16, tag="wo")

        nc.gpsimd.memset(ones_t[:], 1.0)
        nc.gpsimd.memset(eps_t[:], float(EPS))

        pj = ctx.enter_context(tc.tile_pool(name="proj", bufs=1))
        psum = ctx.enter_context(tc.tile_pool(name="psum", bufs=1, space="PSUM"))
        at = ctx.enter_context(tc.tile_pool(name="att", bufs=2))
        ep = ctx.enter_context(tc.tile_pool(name="esb", bufs=2))
        osb = ctx.enter_context(tc.tile_pool(name="osb", bufs=2))

        xtb = pj.tile([128, MC, N], BF16, tag="xtb")
        wqb = pj.tile([128, MC, HPC * HD], BF16, tag="wqb")
        wkb = pj.tile([128, MC, HPC * HD], BF16, tag="wkb")
        wvb = pj.tile([128, MC, HPC * HD], BF16, tag="wvb")
        # load X^T at per-(mc, seq-chunk) granularity.  The first DMA wave
        # interleaves exactly what the first K-projection group consumes (X^T
        # mc-chunk paired with the matching Wk chunk, in mc order) so the PE
        # can start on mc=0 while later chunks stream; Wv then Wq follow in
        # the order the V/Q projection groups reach them.
        # the first K-projection wave issues from the (startup-idle) Act and
        # DVE sequencers in parallel with Sync issuing the rest — the Sync
        # engine's ~610ns per-issue rate was pacing the kernel start
        for mc in range(MC):
            if mc < 2:
                # halves on separate DMA queues: first quantum's operands
                # arrive in half the time
                nc.scalar.dma_start(xtb[:, mc, 0:256], xt_d[:, mc, 0:256])
                nc.scalar.dma_start(xtb[:, mc, 256:512], xt_d[:, mc, 256:512])
                nc.sync.dma_start(wkb[:, mc, 0:256], wk_d[:, mc, 0:256])
                nc.sync.dma_start(wkb[:, mc, 256:512], wk_d[:, mc, 256:512])
            else:
                nc.scalar.dma_start(xtb[:, mc, 0:512], xt_d[:, mc, 0:512])
                nc.sync.dma_start(wkb[:, mc, :], wk_d[:, mc, :])
        for mc in range(MC):
            nc.sync.dma_start(wvb[:, mc, :], wv_d[:, mc, :])
        for mc in range(MC):
            nc.sync.dma_start(wqb[:, mc, :], wq_d[:, mc, :])
        nc.sync.dma_start(msk_t[:], msk_d[:, 0, 0:128])
        nc.sync.dma_start(lam_t[:], lam_d[:])
        # round-1 X^T ahead of wo: the bulk proj(1) quanta at round 0's end
        # are the earliest consumer; wo isn't needed until outproj(0)
        for mc in range(MC):
            nc.scalar.dma_start(xtb[:, mc, 512:1024], xt_d[:, mc, 512:1024])
        nc.sync.dma_start(wo_t[:], wo_d[:])
        for qch in range(2, QC):
            for mc in range(MC):
                nc.sync.dma_start(xtb[:, mc, qch * 512:(qch + 1) * 512],
                                  xt_d[:, mc, qch * 512:(qch + 1) * 512])

        # PSUM rings: 8 banks total = sg 2x2 + pp 2x1 + o1 + o2.
        PSB = {"sg": 2, "pp": 2}

        def drain(dst, src):
            # all PSUM drains on the DVE: the Act engine is the busier of
            # the two (it owns every exp), so it keeps only activations
            nc.vector.tensor_copy(dst, src)

        # ---- projection / out-projection work, chopped into "quanta" of
        # two 512-wide matmuls each.  Quanta are interleaved into the
        # attention kc loops as PE filler: the attention stream is paced by
        # the Act exp pipeline (one [128,1024] exp per k-chunk takes ~1040ns
        # vs ~810ns of PE work), so without filler the PE idles at every
        # sg-slot rotation.  tag selects the PSUM ring ("pp" during
        # attention, "sg" only when the attention stream is quiet).

        def proj_v_half(r, sc, tag):
            st = {}

            def quantum(mcp, st=st, sc=sc, tag=tag):
                if mcp == 0:
                    st['ps'] = psum.tile([128, 512], F32, tag=tag,
                                         name="pjv", bufs=PSB[tag])
                ps = st['ps']
                for mc in (2 * mcp, 2 * mcp + 1):
                    nc.tensor.matmul(
                        ps[:], xtb[:, mc, sc * 128:(sc + 1) * 128],
                        wvb[:, mc, :],
                        start=(mc == 0), stop=(mc == MC - 1))
                if mcp == 3:
                    drain(vb[:, sc, :], ps[:])

            return [lambda m=m: quantum(m) for m in range(4)]

        def proj_qk_half(r, wsrc, dst, hh, tag):
            st = {}

            def quantum(mcp, st=st, r=r, hh=hh, tag=tag):
                if mcp == 0:
                    st['ps'] = psum.tile([128, 512], F32, tag=tag,
                                         name="pjqk", bufs=PSB[tag])
                ps = st['ps']
                for mc in (2 * mcp, 2 * mcp + 1):
                    nc.tensor.matmul(
                        ps[:], wsrc[:, mc, hh * HD:(hh + 1) * HD],
                        xtb[:, mc, r * 512:(r + 1) * 512],
                        start=(mc == 0), stop=(mc == MC - 1))
                if mcp == 3:
                    drain(dst[:, r, hh, :], ps[:])

            return [lambda m=m: quantum(m) for m in range(4)]

        def proj_quanta(r, tags):
            # K first: the attention S-matmuls' stationaries (kt) are the
            # first thing round r's heads consume; V next (PV stationaries);
            # Q (the S moving operand, needed per-head) last.
            # Returns a list of GROUPS (each one PSUM tenancy).
            units = ([('k', hh) for hh in range(HPC)]
                     + [('v', KCQ * r + i) for i in range(KCQ)]
                     + [('q', hh) for hh in range(HPC)])
            out = []
            for ui, (kind, a) in enumerate(units):
                tag = tags[ui % len(tags)]
                if kind == 'k':
                    out.append(proj_qk_half(r, wkb, kt, a, tag))
                elif kind == 'v':
                    out.append(proj_v_half(r, a, tag))
                else:
                    out.append(proj_qk_half(r, wqb, qt, a, tag))
            return out

        def outproj_half(rr, oc, tag):
            # output chunk oc of round rr, accumulated over the four heads;
            # quantum = two head matmuls.
            st = {}

            def quantum(hp, st=st, rr=rr, oc=oc, tag=tag):
                if hp == 0:
                    st['ps'] = psum.tile([128, 512], F32, tag=tag,
                                         name="ops", bufs=PSB[tag])
                ps = st['ps']
                for h in (2 * hp, 2 * hp + 1):
                    nc.tensor.matmul(
                        ps[:], wo_t[:, h, oc, :],
                        otf[h][:, rr * 512:(rr + 1) * 512],
                        start=(h == 0), stop=(h == HPC - 1))
                if hp == 1:
                    ob = osb.tile([128, 512], BF16, tag="ob")
                    drain(ob[:], ps[:])
                    nc.sync.dma_start(
                        out_d[oc * 128:(oc + 1) * 128,
                              rr * 512:(rr + 1) * 512],
                        ob[:])

            return [lambda m=m: quantum(m) for m in range(2)]

        class Filler:
            """Dribbles queued quanta (grouped by PSUM tenancy) into the
            attention emission at a fixed per-kc-event rate; boost() places
            a few right before known stall points; finish_group() completes
            the open group so the pp ring has no long-lived tenant when the
            softmax-sum tiles claim their slots; flush() emits the rest."""

            def __init__(self, groups, rate):
                self.groups = [list(g) for g in groups]
                self.gi = 0
                self.qi = 0
                self.rate = rate
                self.credit = 0.0

            def _pop1(self):
                while self.gi < len(self.groups):
                    g = self.groups[self.gi]
                    if self.qi < len(g):
                        q = g[self.qi]
                        self.qi += 1
                        q()
                        return True
                    self.gi += 1
                    self.qi = 0
                return False

            def tick(self):
                self.credit += self.rate
                while self.credit >= 1.0 and self._pop1():
                    self.credit -= 1.0
                self.credit = min(self.credit, 4.0)

            def boost(self, n):
                for _ in range(n):
                    if not self._pop1():
                        break

            def finish_group(self):
                if self.gi < len(self.groups) and self.qi > 0:
                    g = self.groups[self.gi]
                    while self.qi < len(g):
                        g[self.qi]()
                        self.qi += 1
                    self.gi += 1
                    self.qi = 0

            def flush(self):
                while self._pop1():
                    pass

        def attn_head(qc, h, tick, boost, finish):
            nkc = KCQ * qc + KCQ  # k chunks in play
            q0 = qc * 512
            # filler ahead of the head's first S matmuls: they wait for the
            # previous head's exp/epilogue to free their PSUM slots, and the
            # in-order PE can't pull later work past them
            boost(3)
            o1 = psum.tile([128, 512], F32, tag="o1", name="o1")
            o2 = psum.tile([128, 512], F32, tag="o2", name="o2")
            pending = []
            # deferred softmax-sum contributions: (bf16 tile ap, w0) per
            # comp; full chunks fold pairwise then pairs into quads on the
            # DVE, diagonal chunks contribute directly.
            sums = ([], [])
            pair_hold = [None]  # e tile awaiting its pair partner
            quad_hold = [None]  # pair tile awaiting its quad partner

            def emit_pv(item):
                e, kc, w0 = item
                st = (kc == 0)
                sp = (kc == nkc - 1)
                nc.tensor.matmul(
                    o1[:, w0:512], vb[:, kc, h * HD:(h + 1) * HD],
                    e[:, 0, w0:512], start=st, stop=sp)
                nc.tensor.matmul(
                    o2[:, w0:512], vb[:, kc, h * HD:(h + 1) * HD],
                    e[:, 1, w0:512], start=st, stop=sp)

            for kc in range(nkc):
                j = kc - KCQ * qc
                w0 = max(0, 128 * j)  # first valid col of chunk
                # Both components of one k-chunk go to one fresh PSUM tile,
                # emitted back-to-back: the stationaries sit in disjoint PE
                # row groups (partitions 0:64 / 64:128), so the hardware runs
                # the two 64-contraction matmuls CONCURRENTLY (row tiling) —
                # the pair spans ~one matmul duration, not two.
                ps = psum.tile([128, 2, 512], F32, tag="sg", name="s12",
                               bufs=PSB["sg"])
                qcc, c0 = kc // KCQ, (kc % KCQ) * 128
                nc.tensor.matmul(
                    ps[:, 0, w0:512], kt[0:64, qcc, h, c0:c0 + 128],
                    qt[0:64, qc, h, w0:512], start=True, stop=True)
                nc.tensor.matmul(
                    ps[:, 1, w0:512], kt[64:128, qcc, h, c0:c0 + 128],
                    qt[64:128, qc, h, w0:512], start=True, stop=True)
                e = ep.tile([128, 2, 512], BF16, tag="e", name="e", bufs=10)
                if j >= 2:
                    # narrow chunk: skip the dead prefix on the Act engine
                    # (two contiguous activations — a single strided-AP one
                    # measured ~45% slower on hardware)
                    nc.scalar.activation(
                        e[:, 0, w0:512], ps[:, 0, w0:512],
                        AF.Exp, scale=float(SCALING))
                    nc.scalar.activation(
                        e[:, 1, w0:512], ps[:, 1, w0:512],
                        AF.Exp, scale=float(SCALING))
                else:
                    nc.scalar.activation(
                        e[:].rearrange("p a b -> p (a b)"),
                        ps[:].rearrange("p a b -> p (a b)"),
                        AF.Exp, scale=float(SCALING))
                if j >= 0:  # triangle mask on the diagonal block
                    for c in (0, 1):
                        nc.vector.tensor_mul(
                            e[:, c, w0:w0 + 128], e[:, c, w0:w0 + 128],
                            msk_t[:, 0:128])
                    sums[0].append((e[:, 0, w0:512], w0))
                    sums[1].append((e[:, 1, w0:512], w0))
                elif pair_hold[0] is None:
                    pair_hold[0] = e
                else:
                    e_prev, pair_hold[0] = pair_hold[0], None
                    pr = ep.tile([128, 2, 512], BF16, tag="pr", name="pr",
                                 bufs=4)
                    nc.vector.tensor_add(pr[:, 0, :], e_prev[:, 0, :],
                                         e[:, 0, :])
                    nc.vector.tensor_add(pr[:, 1, :], e_prev[:, 1, :],
                                         e[:, 1, :])
                    if quad_hold[0] is None:
                        quad_hold[0] = pr
                    else:
                        pr_prev, quad_hold[0] = quad_hold[0], None
                        qd = ep.tile([128, 2, 512], BF16, tag="qd", name="qd",
                                     bufs=3)
                        nc.vector.tensor_add(qd[:, 0, :], pr_prev[:, 0, :],
                                             pr[:, 0, :])
                        nc.vector.tensor_add(qd[:, 1, :], pr_prev[:, 1, :],
                                             pr[:, 1, :])
                        sums[0].append((qd[:, 0, :], 0))
                        sums[1].append((qd[:, 1, :], 0))
                pending.append((e, kc, w0))
                if len(pending) > 6:
                    emit_pv(pending.pop(0))
                tick()
            if quad_hold[0] is not None:
                pr, quad_hold[0] = quad_hold[0], None
                sums[0].append((pr[:, 0, :], 0))
                sums[1].append((pr[:, 1, :], 0))
            if pair_hold[0] is not None:
                e_left, pair_hold[0] = pair_hold[0], None
                sums[0].append((e_left[:, 0, :], 0))
                sums[1].append((e_left[:, 1, :], 0))
            while pending:
                emit_pv(pending.pop(0))
            # filler right before the sum matmuls: their moving operands
            # come off the exp/fold pipeline, so the PE otherwise stalls
            boost(2)
            # close any open filler group, then the softmax-sum tiles take
            # the two pp slots (keeping both sg slots for the S stream)
            finish()
            ssum1 = psum.tile([128, 512], F32, tag="pp", name="ssum1",
                              bufs=PSB["pp"])
            ssum2 = psum.tile([128, 512], F32, tag="pp", name="ssum2",
                              bufs=PSB["pp"])
            for half, contribs in enumerate(sums):
                dst = ssum1 if half == 0 else ssum2
                for ci, (src, w0c) in enumerate(contribs):
                    nc.tensor.matmul(dst[:, w0c:512], ones_t[:], src,
                                     start=(ci == 0),
                                     stop=(ci == len(contribs) - 1))
            # ---- epilogue: d = o1/g - (lam/g)*(s1/s2)*o2; the s1/g
            # column scale cancels in the RMS norm.  |lam/g| <= 1 keeps
            # d^2 inside bf16 range.  Division via fast DVE reciprocal.
            r2 = at.tile([128, 512], F32, tag="r2")
            nc.vector.reciprocal_approx_fast(out=r2[:], in_=ssum2[:])
            w = at.tile([128, 512], F32, tag="w")
            nc.vector.scalar_tensor_tensor(
                w[:], ssum1[:], lam_t[:, h:h + 1], r2[:],
                ALU.mult, ALU.mult)
            t = at.tile([128, 512], F32, tag="t", bufs=1)
            nc.vector.tensor_mul(t[:], o2[:], w[:])
            d = at.tile([128, 512], BF16, tag="d")
            nc.vector.scalar_tensor_tensor(
                d[:], o1[:], lam_t[:, HPC + h:HPC + h + 1], t[:],
                ALU.mult, ALU.subtract)
            osq = at.tile([128, 512], BF16, tag="osq")
            nc.vector.tensor_mul(osq[:], d[:], d[:])
            # the o1 bank is free right here (d has consumed it)
            ssq = psum.tile([128, 512], F32, tag="o1", name="ssq")
            nc.tensor.matmul(ssq[:], ones_t[:], osq[:],
                             start=True, stop=True)
            lnv = at.tile([128, 512], F32, tag="lnv", bufs=1)
            nc.scalar.activation(lnv[:], ssq[:], AF.Ln,
                                 scale=float(1.0 / HD), bias=eps_t[:])
            rr = at.tile([128, 512], BF16, tag="rr")
            nc.scalar.activation(rr[:], lnv[:], AF.Exp, scale=-0.5)
            nc.vector.tensor_mul(otf[h][:, q0:q0 + 512], d[:], rr[:])

        # ---- emission schedule: round 0's projections run upfront; after
        # that, round qc+1's projection quanta are dribbled INTO round qc's
        # attention loops (the attention stream is Act-paced, so the PE has
        # slack there), out-projections for rounds 0/1 go at their round
        # ends, round 2's out-projection fills round 3's attention, and
        # round 3's out-projection is the tail.
        for g in proj_quanta(0, ("pp",)):
            for q in g:
                q()
        for qc in range(QC):
            events = HPC * (KCQ * qc + KCQ)
            if qc < QC - 1:
                # round 0: the round-1 X^T chunks are still in flight on the
                # DMA queues during round-0 attention, so dribbling proj(1)
                # quanta in would stall the PE on data — emit them all in
                # the round-end bulk instead.  The uniform rate leaves ~20
                # quanta per round for the targeted stall-point boosts.
                rate = 0.0 if qc == 0 else 28.0 / events
                fill = Filler(proj_quanta(qc + 1, ("pp",)), rate)
            else:
                # boosts alone place the round-2 out-projection quanta at
                # the stall points
                fill = Filler([outproj_half(3 - 1, oc, "pp")
                               for oc in range(8)], 0.0)
            for h in range(HPC):
                # round 0, head 0 is the only window where filler data
                # (round-1 X^T) hasn't landed yet — no boosts there
                boost = ((lambda n: None) if (qc == 0 and h == 0)
                         else fill.boost)
                attn_head(qc, h, fill.tick, boost, fill.finish_group)
            fill.flush()
            if qc < QC - 2:
                for oc in range(8):
                    for q in outproj_half(qc, oc, "pp"):
                        q()
            elif qc == QC - 1:
                for oc in range(8):
                    for q in outproj_half(qc, oc, "pp"):
                        q()

    nc.compile()
    return nc


def _prep_inputs(X, Wq, Wk, Wv, Wo, lambda_q1, lambda_k1, lambda_q2,
                 lambda_k2, rms_scale):
    f32 = np.float32
    bf16 = ml_dtypes.bfloat16
    X = np.asarray(X, f32)
    Wq = np.asarray(Wq, f32)
    Wk = np.asarray(Wk, f32)
    Wv = np.asarray(Wv, f32)
    Wo = np.asarray(Wo, f32)
    lam = (np.exp(np.sum(np.asarray(lambda_q1, f32) * np.asarray(lambda_k1, f32), -1))
           - np.exp(np.sum(np.asarray(lambda_q2, f32) * np.asarray(lambda_k2, f32), -1))
           + f32(LAMBDA_INIT)).astype(f32)  # [H]
    # fold rms_scale and (1-lambda_init) into Wo
    wo_f = (Wo.reshape(H, HD, D_MODEL)
            * np.asarray(rms_scale, f32)[None, :, None]
            * f32(1.0 - LAMBDA_INIT)).astype(f32)

    # causal masks for the 4 diagonal-region chunk offsets
    msk = np.zeros((128, KCQ, 512), f32)
    kk = np.arange(128)[:, None]
    cc = np.arange(512)[None, :]
    for j in range(KCQ):
        msk[:, j, :] = (cc >= 128 * j + kk).astype(f32)

    in_maps = []
    for c in range(8):
        b, hg = divmod(c, 4)
        xt = X[b].T.reshape(MC, 128, N).transpose(1, 0, 2)  # [128, MC, N]
        sl = slice(hg * HPC * HD, (hg + 1) * HPC * HD)
        wq = Wq[:, sl].reshape(MC, 128, HPC * HD).transpose(1, 0, 2)
        wk = Wk[:, sl].reshape(MC, 128, HPC * HD).transpose(1, 0, 2)
        wv = Wv[:, sl].reshape(MC, 128, HPC * HD).transpose(1, 0, 2)
        wo = wo_f[hg * HPC:(hg + 1) * HPC].reshape(HPC, HD, 8, 128).transpose(1, 0, 2, 3)
        lv = lam[hg * HPC:(hg + 1) * HPC]
        g = np.maximum(np.abs(lv), f32(1.0)).astype(f32)
        lam_row = np.concatenate([lv / g, 1.0 / g]).astype(f32)
        lam_bc = np.broadcast_to(lam_row[None, :], (128, 2 * HPC))
        in_maps.append({
            "xt": np.ascontiguousarray(xt).astype(bf16),
            "wq": np.ascontiguousarray(wq).astype(bf16),
            "wk": np.ascontiguousarray(wk).astype(bf16),
            "wv": np.ascontiguousarray(wv).astype(bf16),
            "wo": np.ascontiguousarray(wo).astype(bf16),
            "lam": np.ascontiguousarray(lam_bc.astype(f32)),
            "msk": msk.astype(bf16),
        })
    return in_maps


def kernel(X, Wq, Wk, Wv, Wo, lambda_q1, lambda_k1, lambda_q2, lambda_k2,
           rms_scale, _trace=False):
    if "nc" not in _cache:
        _cache["nc"] = _build()
    nc = _cache["nc"]
    in_maps = _prep_inputs(X, Wq, Wk, Wv, Wo, lambda_q1, lambda_k1,
                           lambda_q2, lambda_k2, rms_scale)
    res = run_bass_kernel_spmd(nc, in_maps, list(range(8)), trace=_trace)
    out = np.zeros((B, N, D_MODEL), np.float32)
    for c in range(8):
        b = c // 4
        out[b] += res.results[c]["outT"].T.astype(np.float32)
    _cache["last_exec_ns"] = res.exec_time_ns
    _cache["last_res"] = res
    return out

